# revision 18
# baseline (speedup 1.0000x reference)
"""ContxE temporal KG scoring kernel v2 for Trainium2 (Bass/Tile).

Transposed-layout redesign of the baseline:
 - h/t/r embedding rows are gathered (indirect DMA) then DMA-transposed once
   into [d-partition, element] layout; ALL elementwise math runs there, so no
   per-product transposes are needed (baseline moved 16MB/core through
   dma_start_transpose; this moves 10MB).
 - cos/sin of time_table rows are computed on-chip: theta = d*inc01 via a K=1
   outer-product matmul, then ACT Sin with per-partition(=per-d) bias. No cs
   gather at all.
 - logits contract over d on the Tensor engine directly from the transposed
   UV products; softmax stays in [w, e] layout using the re-exp trick:
   Ex=exp(L); Z=ones@Ex; L+=(-1)*ln(Z) via a K=1 accumulating matmul; exp
   again -> normalized alphas. No PE transposes, no reciprocal.
 - G vectors (attention-weighted window rotations) come from K=5 matmuls of
   alpha against constant [5,128] tables; F = sum(AB*G)+r ; out = colsum |F|
   via ones-matmul.
 - clamped elements (d<4, ~1%/core) are permuted host-side into a final
   half-supertile, class-sorted into fixed 32-column ranges; constant-delta
   matmuls (dwlog, dcsmH) correct their logits and G vectors. Fixed ranges
   keep the instruction stream identical across cores (SPMD) and runs.

Per-core layout: 2176 slots = 8 supertiles of E=256 (normals + duplicate
padding) + 1 half supertile of E=128 (4 classes x 32 slots; real clamped
elements first, duplicate padding after; padding results are discarded).
"""

import sys

if "/opt/trn_rl_repo" not in sys.path:
    sys.path.insert(0, "/opt/trn_rl_repo")

import numpy as np
import ml_dtypes

import concourse.bass as bass
import concourse.bacc as bacc
import concourse.tile as tile
from concourse import mybir
from concourse.bass_utils import run_bass_kernel_spmd
from concourse._compat import with_exitstack

N_CORES = 8
B = 16384
BL = B // N_CORES          # 2048
D = 512
NCH = 4                    # chunks of 128 in D
W = 5
N_ENTITY = 100000
N_RELATION = 256
N_DAY = 365
E0 = 256                   # full supertile width
NSLOT = 2176               # 8*256 + 128
NGT = NSLOT // 128         # 17 gather tiles

F32 = mybir.dt.float32
BF16 = mybir.dt.bfloat16
I32 = mybir.dt.int32

AF = mybir.ActivationFunctionType
OP = mybir.AluOpType
IOA = bass.IndirectOffsetOnAxis

# product p -> (fam, vec(0=ck,1=sk), sign)
PW = [(0, 0, 1.0), (0, 1, 1.0), (1, 0, 1.0), (1, 1, 1.0),
      (2, 1, -1.0), (2, 0, 1.0), (3, 1, -1.0), (3, 0, 1.0)]
# G slot groups (half, slot) -> (fam, vec, sign); slots follow (Ah,Bh,At,Bt)
GRPS = [[(0, 0, 1.0), (0, 1, 1.0), (1, 0, -1.0), (1, 1, -1.0)],
        [(2, 1, -1.0), (2, 0, 1.0), (3, 1, -1.0), (3, 0, 1.0)]]


@with_exitstack
def _emit(ctx, tc, outs, ins):
    nc = tc.nc
    embE = ins["embE"]         # [N_ENTITY, 2D] bf16
    embR = ins["embR"]         # [N_RELATION, 2D] bf16
    out = outs["out"]          # [1, NSLOT] f32

    singles = ctx.enter_context(tc.tile_pool(name="singles", bufs=1))
    gpool = ctx.enter_context(tc.tile_pool(name="g", bufs=2))
    tpool = ctx.enter_context(tc.tile_pool(name="tp", bufs=2))
    wk = ctx.enter_context(tc.tile_pool(name="wk", bufs=1))
    wk2 = ctx.enter_context(tc.tile_pool(name="wk2", bufs=2))
    csp = ctx.enter_context(tc.tile_pool(name="csp", bufs=2))
    sm = ctx.enter_context(tc.tile_pool(name="sm", bufs=2))
    ppA = ctx.enter_context(tc.tile_pool(name="ppA", bufs=1, space="PSUM"))
    ppL = ctx.enter_context(tc.tile_pool(name="ppL", bufs=2, space="PSUM"))
    ppG = ctx.enter_context(tc.tile_pool(name="ppG", bufs=2, space="PSUM"))
    ppO = ctx.enter_context(tc.tile_pool(name="ppO", bufs=1, space="PSUM"))

    # resident constants (DMA'd from DRAM)
    sb_h = singles.tile([128, NGT], I32)
    sb_t = singles.tile([128, NGT], I32)
    sb_r = singles.tile([128, NGT], I32)
    sb_w = singles.tile([128, 8, 4, 5], BF16)
    sb_cs = singles.tile([128, 4, 4, 128], BF16)   # combo (vec,sign), chunk
    sb_dw = singles.tile([128, 8, 4, 4, 5], BF16)
    sb_dcs = singles.tile([128, 4, 4, 4, 128], BF16)
    sb_inc = singles.tile([128, 4], F32)
    sb_dr = singles.tile([128, NSLOT], F32)
    sb_bc = singles.tile([128, 4], F32)
    sb_bs = singles.tile([128, 4], F32)
    bo4 = singles.tile([128, 4], BF16)     # Z summer: [32f+w, f] = 1
    brep = singles.tile([4, 128], F32)     # replicator: [f, 32f+w] = 1
    zz1 = singles.tile([1, 128], BF16)     # zeros (PSUM bank clear matmul)
    zrow = singles.tile([1, E0], BF16)
    ones128 = singles.tile([128, 1], BF16)
    orow = singles.tile([1, NSLOT], F32)

    # index tiles + small constants on the sync queue (it also issues the
    # transposes, so keep it short); bulk tables go via the scalar queue
    for name, t in [("h_idx", sb_h), ("t_idx", sb_t), ("r_idx", sb_r),
                    ("zz1", zz1), ("zrow", zrow), ("ones128", ones128),
                    ("bo4", bo4), ("brep", brep)]:
        nc.sync.dma_start(t[:], ins[name][:])
    for name, t in [("inc01T", sb_inc), ("biasC", sb_bc), ("biasS", sb_bs),
                    ("wlog", sb_w), ("csmH", sb_cs), ("dRep", sb_dr),
                    ("dwlog", sb_dw), ("dcsmH", sb_dcs)]:
        nc.scalar.dma_start(t[:], ins[name][:])

    def supertile(s, E):
        nj = E // 128
        c0 = 2048 if s == 8 else s * E0   # starting slot/column
        # ---- gathers (row-major, staged in j-pairs) + transpose to [d, e]
        gT = tpool.tile([128, 16, E0], BF16, tag="gT")
        rgT = tpool.tile([128, 8, E0], BF16, tag="rgT")
        for jp in range((nj + 1) // 2):
            graw = gpool.tile([128, 2, 2, 1024], BF16, tag="graw")
            rgraw = gpool.tile([128, 2, 1024], BF16, tag="rgraw")
            for jj in range(min(2, nj - 2 * jp)):
                j = 2 * jp + jj
                gt = c0 // 128 + j        # gather tile column
                nc.gpsimd.indirect_dma_start(
                    out=graw[:, jj, 0, :], out_offset=None, in_=embE[:],
                    in_offset=IOA(ap=sb_h[:, gt:gt + 1], axis=0))
                nc.gpsimd.indirect_dma_start(
                    out=graw[:, jj, 1, :], out_offset=None, in_=embE[:],
                    in_offset=IOA(ap=sb_t[:, gt:gt + 1], axis=0))
                nc.gpsimd.indirect_dma_start(
                    out=rgraw[:, jj, :], out_offset=None, in_=embR[:],
                    in_offset=IOA(ap=sb_r[:, gt:gt + 1], axis=0))
                nc.sync.dma_start_transpose(
                    gT[:, :, 128 * j:128 * (j + 1)],
                    graw[:, jj, :, :].rearrange("p a b -> p (a b)"))
                nc.sync.dma_start_transpose(
                    rgT[:, :, 128 * j:128 * (j + 1)], rgraw[:, jj, :])
        # ---- cos/sin: Sin(d*inc01 + base) via ACT per-partition scale/bias
        csT = csp.tile([128, 8, E0], BF16, tag="csT")
        dr = sb_dr[:, c0:c0 + E]
        for c in range(4):
            nc.scalar.activation(csT[:, c, 0:E], dr, AF.Sin,
                                 scale=sb_inc[:, c:c + 1],
                                 bias=sb_bc[:, c:c + 1])
            nc.scalar.activation(csT[:, 4 + c, 0:E], dr, AF.Sin,
                                 scale=sb_inc[:, c:c + 1],
                                 bias=sb_bs[:, c:c + 1])
        # ---- ph products and A/B fold ----
        phA = wk.tile([128, 2, 8, E0], BF16, tag="phA")
        phB = wk.tile([128, 2, 8, E0], BF16, tag="phB")
        for ht in range(2):
            g8 = gT[:, 8 * ht:8 * ht + 8, 0:E]
            nc.vector.tensor_tensor(out=phA[:, ht, :, 0:E], in0=g8,
                                    in1=csT[:, 0:8, 0:E], op=OP.mult)
            nc.vector.tensor_tensor(out=phB[:, ht, 0:4, 0:E],
                                    in0=gT[:, 8 * ht:8 * ht + 4, 0:E],
                                    in1=csT[:, 4:8, 0:E], op=OP.mult)
            nc.vector.tensor_tensor(out=phB[:, ht, 4:8, 0:E],
                                    in0=gT[:, 8 * ht + 4:8 * ht + 8, 0:E],
                                    in1=csT[:, 0:4, 0:E], op=OP.mult)
        # abT dims (ht, a/b, chunk, e): slot s4 = (Ah, Bh, At, Bt) = ht*2+ab
        abT = wk2.tile([128, 2, 2, 4, E0], BF16, tag="abT")
        nc.vector.tensor_tensor(out=abT[:, :, 0, :, 0:E],
                                in0=phA[:, :, 0:4, 0:E],
                                in1=phA[:, :, 4:8, 0:E], op=OP.subtract)
        nc.vector.tensor_tensor(out=abT[:, :, 1, :, 0:E],
                                in0=phB[:, :, 0:4, 0:E],
                                in1=phB[:, :, 4:8, 0:E], op=OP.add)
        # ---- UV products: uvT[p] = rg(r/i) * AB(slot p%4) ----
        uvT = wk.tile([128, 8, 4, E0], BF16, tag="uvT")
        for p in range(8):
            ri = p // 4
            s4 = p % 4
            nc.vector.tensor_tensor(
                out=uvT[:, p, :, 0:E], in0=rgT[:, 4 * ri:4 * ri + 4, 0:E],
                in1=abT[:, s4 // 2, s4 % 2, :, 0:E], op=OP.mult)
        # ---- logits: Lp32 [128, E], fam f in rows 32f..32f+5 (col-tiled)
        Lp = ppL.tile([128, E0], F32, tag="Lp")
        nc.tensor.matmul(Lp[:, 0:E], zz1[:], zrow[:, 0:E], start=True,
                         stop=False, skip_group_check=True)
        mms = []
        for f in range(4):
            for pp in range(2):
                p = 2 * f + pp
                for c in range(NCH):
                    mms.append((f, sb_w[:, p, c, :], uvT[:, p, c, 0:E], None))
            if E == 128:  # fix half-supertile: class delta MMs
                for pp in range(2):
                    p = 2 * f + pp
                    for c in range(NCH):
                        for k in range(4):
                            mms.append((f, sb_dw[:, p, c, k, :],
                                        uvT[:, p, c, 32 * k:32 * k + 32],
                                        32 * k))
        for i, (f, lhs, rhs, rs) in enumerate(mms):
            o = (Lp[32 * f:32 * f + 5, 0:E] if rs is None
                 else Lp[32 * f:32 * f + 5, rs:rs + 32])
            nc.tensor.matmul(o, lhs, rhs, start=False,
                             stop=(i == len(mms) - 1),
                             skip_group_check=True,
                             tile_position=(0, 32 * f))
        # ---- softmax: exp, Z per fam, reciprocal, replicate, multiply ----
        Ex = sm.tile([128, E0], BF16, tag="Ex")
        nc.scalar.activation(Ex[:, 0:E], Lp[:, 0:E], AF.Exp)
        thz = ppA.tile([128, 2, E0], F32, tag="th")
        nc.tensor.matmul(thz[0:4, 1, 0:E], bo4[:], Ex[:, 0:E],
                         start=True, stop=True, skip_group_check=True)
        rcZ = sm.tile([4, E0], F32, tag="rcZ")
        nc.vector.reciprocal_approx_fast(rcZ[:, 0:E], thz[0:4, 1, 0:E])
        nc.tensor.matmul(thz[:, 0, 0:E], brep[:], rcZ[:, 0:E],
                         start=True, stop=True, skip_group_check=True)
        al = sm.tile([128, E0], BF16, tag="al")
        nc.vector.tensor_tensor(out=al[:, 0:E], in0=Ex[:, 0:E],
                                in1=thz[:, 0, 0:E], op=OP.mult)
        # qall shares uvT's buffer (same shape, disjoint lifetime)
        qall = wk.tile([128, 8, 4, E0], BF16, tag="uvT")
        for c in range(NCH):
            for half in range(2):
                G4 = ppG.tile([128, 4, E0], F32, tag="G4")
                for sl in range(4):
                    f, v, sg = GRPS[half][sl]
                    cmb = 2 * v + (0 if sg > 0 else 1)
                    fb = slice(32 * f, 32 * f + 5)
                    mms = [(sb_cs[fb, cmb, c, :], al[fb, 0:E], None)]
                    if E == 128:
                        for k in range(4):
                            mms.append((sb_dcs[fb, cmb, c, k, :],
                                        al[fb, 32 * k:32 * k + 32], 32 * k))
                    # slots 0-1 / 2-3 pair up per 2KB PSUM bank
                    for i, (lhs, rhs, rs) in enumerate(mms):
                        o = (G4[:, sl, 0:E] if rs is None
                             else G4[:, sl, rs:rs + 32])
                        nc.tensor.matmul(o, lhs, rhs,
                                         start=(i == 0 and sl % 2 == 0),
                                         stop=(i == len(mms) - 1 and sl % 2 == 1),
                                         skip_group_check=True,
                                         tile_position=(32 * f, 0))
                g4sb = sm.tile([128, 4, E0], BF16, tag="g4sb")
                nc.scalar.activation(g4sb[:, :, 0:E], G4[:, :, 0:E], AF.Copy)
                # qall layout [128, (half*4+slot), chunk, e]
                nc.vector.tensor_tensor(
                    out=qall[:, 4 * half:4 * half + 4, c, 0:E],
                    in0=abT[:, :, :, c, 0:E].rearrange("p a b e -> p (a b) e"),
                    in1=g4sb[:, :, 0:E], op=OP.mult)
        # ---- folds, abs, colsum ----
        # fold1: (Ah+At), (Bh+Bt) per half -> t1 [128, half(2), 2, 4, E]
        t1 = wk.tile([128, 2, 2, 4, E0], BF16, tag="t1")
        for half in range(2):
            nc.vector.tensor_tensor(
                out=t1[:, half, :, :, 0:E],
                in0=qall[:, 4 * half:4 * half + 2, :, 0:E],
                in1=qall[:, 4 * half + 2:4 * half + 4, :, 0:E], op=OP.add)
        t2 = wk.tile([128, 2, 4, E0], BF16, tag="t2")
        nc.vector.tensor_tensor(out=t2[:, :, :, 0:E],
                                in0=t1[:, :, 0, :, 0:E],
                                in1=t1[:, :, 1, :, 0:E], op=OP.add)
        Fa = wk.tile([128, 2, 4, E0], BF16, tag="Fa")
        for half in range(2):
            Fv = wk.tile([128, 4, E0], BF16, tag=f"Fv{half}")
            nc.vector.tensor_tensor(out=Fv[:, :, 0:E],
                                    in0=t2[:, half, :, 0:E],
                                    in1=rgT[:, 4 * half:4 * half + 4, 0:E],
                                    op=OP.add)
            nc.scalar.activation(Fa[:, half, :, 0:E], Fv[:, :, 0:E], AF.Abs)
        osum = ppO.tile([1, E0], F32, tag="osum")
        for c in range(NCH):
            for hh in range(2):
                nc.tensor.matmul(osum[:, 0:E], ones128[:], Fa[:, hh, c, 0:E],
                                 start=(c == 0 and hh == 0),
                                 stop=(c == NCH - 1 and hh == 1),
                                 skip_group_check=True)
        nc.vector.tensor_copy(orow[:, c0:c0 + E], osum[:, 0:E])

    # fix half-supertile in the middle: its delta tables load in the
    # background (DMAs emitted late to keep startup bandwidth free) and its
    # serial tail overlaps neighboring supertiles
    for s in [0, 1, 2, 3, 8, 4, 5, 6, 7]:
        supertile(s, E0 if s != 8 else 128)
    nc.sync.dma_start(out[:], orow[:])


def _host_prep(h_i, t_i, r_i, d_i, emb_E_real, emb_E_img, emb_R_real,
               emb_R_img, time_table):
    bf = ml_dtypes.bfloat16
    embE = np.concatenate([emb_E_real, emb_E_img], axis=1).astype(bf)
    embR = np.concatenate([emb_R_real, emb_R_img], axis=1).astype(bf)
    tt = np.asarray(time_table, np.float64)
    base = tt[0]
    inc01 = tt[1] - tt[0]
    ks = np.arange(W)[::-1].astype(np.float64)
    ck = np.cos(ks[:, None] * inc01[None, :])       # [5, D]
    sk = np.sin(ks[:, None] * inc01[None, :])
    ckc = np.tile(ck[None], (4, 1, 1))
    skc = np.tile(sk[None], (4, 1, 1))
    for c in range(4):
        # clamped row 365 = tt[d] + Delta, Delta = (365-d)*inc01:
        # h_real = cos(D)*A - sin(D)*B
        ang = (365.0 - c) * inc01
        for w in range(W):
            if 4 - w > c:
                ckc[c, w] = np.cos(ang)
                skc[c, w] = -np.sin(ang)

    def chunked(v):  # [5, D] -> [4, 128, 5] (c, dd, w)
        return np.ascontiguousarray(v.T.reshape(NCH, 128, W))

    ckch, skch = chunked(ck), chunked(sk)
    wlog = np.empty((128, 8, 4, 5), np.float64)
    dwlog = np.empty((128, 8, 4, 4, 5), np.float64)
    for p, (f, v, sg) in enumerate(PW):
        vn = ckch if v == 0 else skch
        wlog[:, p, :, :] = sg * vn.transpose(1, 0, 2)
        for k in range(4):
            vc = chunked(ckc[k] if v == 0 else skc[k])
            dwlog[:, p, :, k, :] = sg * (vc - vn).transpose(1, 0, 2)
    # csm tables, fam-collapsed: combo = 2*vec + (0 if +1 else 1), with the
    # w-rows replicated into every fam's 32f..32f+5 partition block
    csmH = np.zeros((128, 4, 4, 128), np.float64)
    dcsmH = np.zeros((128, 4, 4, 4, 128), np.float64)
    for v in range(2):
        for si, sg in enumerate([1.0, -1.0]):
            cmb = 2 * v + si
            vn = ck if v == 0 else sk               # [5, D]
            for f in range(4):
                rows = slice(32 * f, 32 * f + 5)
                csmH[rows, cmb, :, :] = sg * vn.reshape(W, NCH, 128)
                for k in range(4):
                    vc = ckc[k] if v == 0 else skc[k]
                    dcsmH[rows, cmb, :, k, :] = (
                        sg * (vc - vn).reshape(W, NCH, 128))
    bo4 = np.zeros((128, 4), np.float64)
    brep = np.zeros((4, 128), np.float64)
    for f in range(4):
        bo4[32 * f:32 * f + 5, f] = 1.0
        brep[f, 32 * f:32 * f + 5] = 1.0
    inc01T = np.ascontiguousarray(inc01.reshape(4, 128).T)
    biasC = np.ascontiguousarray((base + np.pi / 2).reshape(4, 128).T)
    biasS = np.ascontiguousarray(base.reshape(4, 128).T)

    h_i = np.asarray(h_i, np.int64)
    t_i = np.asarray(t_i, np.int64)
    r_i = np.asarray(r_i, np.int64)
    d_i = np.asarray(d_i, np.int64)
    in_maps, scatters = [], []
    for core in range(N_CORES):
        sl = slice(core * BL, (core + 1) * BL)
        hh, tt_, rr, dd = h_i[sl], t_i[sl], r_i[sl], d_i[sl]
        fix = np.where(dd < 4)[0]
        norm = np.where(dd >= 4)[0]
        nfix = len(fix)
        assert nfix <= 128
        slots = np.zeros(NSLOT, np.int64)
        real = np.zeros(NSLOT, bool)
        slots[:len(norm)] = norm
        real[:len(norm)] = True
        slots[len(norm):2048] = norm[0]             # pad
        tail = 2048
        for k in range(4):
            cls = fix[dd[fix] == k]
            assert len(cls) <= 32, f"class {k}: {len(cls)}"
            slots[tail + 32 * k: tail + 32 * k + len(cls)] = cls
            real[tail + 32 * k: tail + 32 * k + len(cls)] = True
            slots[tail + 32 * k + len(cls): tail + 32 * (k + 1)] = norm[0]
        ph, pt, pr, pd = hh[slots], tt_[slots], rr[slots], dd[slots]

        def cols(a):
            return np.ascontiguousarray(a.reshape(NGT, 128).T).astype(np.int32)

        in_maps.append(dict(
            embE=embE, embR=embR,
            h_idx=cols(ph), t_idx=cols(pt), r_idx=cols(pr),
            dRep=np.broadcast_to(pd.astype(np.float32), (128, NSLOT)).copy(),
            wlog=wlog.astype(bf), csmH=csmH.astype(bf),
            dwlog=dwlog.astype(bf), dcsmH=dcsmH.astype(bf),
            inc01T=inc01T.astype(np.float32), biasC=biasC.astype(np.float32),
            biasS=biasS.astype(np.float32),
            bo4=bo4.astype(bf), brep=brep.astype(np.float32),
            zz1=np.zeros((1, 128), bf), zrow=np.zeros((1, E0), bf),
            ones128=np.ones((128, 1), bf),
        ))
        scatters.append((slots, real))
    return in_maps, scatters


_SPECS = dict(
    embE=([N_ENTITY, 2 * D], BF16), embR=([N_RELATION, 2 * D], BF16),
    h_idx=([128, NGT], I32), t_idx=([128, NGT], I32), r_idx=([128, NGT], I32),
    wlog=([128, 8, 4, 5], BF16),
    csmH=([128, 4, 4, 128], BF16), dwlog=([128, 8, 4, 4, 5], BF16),
    dcsmH=([128, 4, 4, 4, 128], BF16), inc01T=([128, 4], F32),
    dRep=([128, NSLOT], F32),
    biasC=([128, 4], F32), biasS=([128, 4], F32), bo4=([128, 4], BF16),
    brep=([4, 128], F32), zz1=([1, 128], BF16), zrow=([1, E0], BF16),
    ones128=([128, 1], BF16),
)


def build_nc():
    nc = bacc.Bacc("TRN2", target_bir_lowering=False, debug=False,
                   enable_asserts=False, num_devices=N_CORES)
    ins = {k: nc.dram_tensor(k, shp, dt, kind="ExternalInput").ap()
           for k, (shp, dt) in _SPECS.items()}
    outs = dict(
        out=nc.dram_tensor("out", [1, NSLOT], F32, kind="ExternalOutput").ap(),
    )
    with tile.TileContext(nc) as tc:
        _emit(tc, outs, ins)
    nc.compile()
    return nc


_NC_CACHE = {}


def kernel(h_i, t_i, r_i, d_i, emb_E_real, emb_E_img, emb_R_real, emb_R_img,
           time_table, _want_results=False, _trace=False):
    in_maps, scatters = _host_prep(h_i, t_i, r_i, d_i, emb_E_real, emb_E_img,
                                   emb_R_real, emb_R_img, time_table)
    if "nc" not in _NC_CACHE:
        _NC_CACHE["nc"] = build_nc()
    nc = _NC_CACHE["nc"]
    res = run_bass_kernel_spmd(
        nc, in_maps, core_ids=list(range(N_CORES)), trace=_trace)
    out = np.empty((B,), np.float32)
    for core in range(N_CORES):
        o = np.asarray(res.results[core]["out"]).astype(np.float32).reshape(NSLOT)
        slots, real = scatters[core]
        oc = np.empty(BL, np.float32)
        oc[slots[real]] = o[real]
        out[core * BL:(core + 1) * BL] = oc
    if _want_results:
        return out, res
    return out
